# revision 44
# baseline (speedup 1.0000x reference)
"""Trainium2 Bass kernel for nn_AttentionLayer (attention pooling over time).

Math (per sample b):
    logits[t] = u . tanh(X[b] @ W)[t]     # (T,)
    att       = softmax_t(logits)
    out[b]    = sum_t att[t] * X[b, t, :] # (D,)

Strategy:
  - Data-parallel over batch across 8 NeuronCores (B=64 -> 8 samples/core).
  - tanh bounds |logit| <= sum|u| < 5, so softmax needs NO max subtraction:
    p[t] = exp(logit[t]) is safe in fp32.  One streaming pass over X with
    PSUM accumulation of sum_t p[t]*x[t]; one divide per sample at the end.
  - The X@W matmul contracts over d, so it needs X^T (d on partitions); the
    weighted sum contracts over t, so it needs X natural (t on partitions).
    The host ships X twice: fp8-e4m3 transposed (8.4 MiB/core) and a
    RESIDUAL-COMPENSATED mixed natural copy (10.5 MiB/core): 3 of every
    4 t-chunks in fp8, the 4th in bf16 carrying its partners' summed fp8
    residuals.  Attention weights are near-uniform (logit std ~0.12), so
    a residual landing on an adjacent timestep keeps ~5/6 of its
    correction: numpy-measured 6.1e-3 total rel err vs >1.6e-2 for naive
    mixed-fp8 (bass matmul has no int8 path; full-fp8 natural is 1.8e-2,
    at the gate).
  - At ~340-355 GB/s effective HBM rate the ~19 MiB is ~56 us of DMA and
    the PE streams are ~57-59 us -- co-designed walls.  The kernel (a) splits
    the slab streams over BOTH HWDGE queues so the xtt stream never
    queues behind the 2x-bigger xn stream (measured mid-pipeline stalls
    were all xtt-sem waits), (b) fills the ~5 us DMA head with warm-up
    matmuls so HAM un-throttles the PE clock (4/8 -> 8/8) before the real
    stream starts (~3.5 us measured cold penalty).
  - X@W runs as ONE fp8 DoubleRow matmul per supertile (K=256, ~241 ns)
    with a SINGLE adaptively-rounded e4m3 W plane: the fp8 rounding
    direction of each W entry is chosen host-side so that (dW @ u) ~ 0
    per row, which kills the tanh-linearized logit error x.(dW u) that
    makes naive one-plane fp8 W unusable (1.44e-2 -> 5.75e-3, equal to
    exact-W).  Beats both bf16-W two-half (2x213 ns) and hi+lo DR
    (2x241 ns) forms.
  - logits (C stage): th [ctx, t] slices as stationary x u moving (N=1),
    4 per supertile; one [128, 8] PSUM group and ONE exp per supertile
    PAIR so the paps pool truly double-buffers; sum_t p on the idle
    Vector engine into per-sample scols.
  - weighted sum (E stage): p columns stationary x natural bf16 slabs
    moving (N=256), lagging the exp stage by 4 supertiles.
  - DMA: all input slabs on the single sync HWDGE ring in need-order
    (each xtt two samples ahead of its xn); u on the scalar ring; out
    stores on gpsimd's idle SWDGE ring (on sync their finalize-gated
    issues blocked the xn stream in FIFO order for tens of us).
  - Measured history (shared host oscillates between clock regimes ~20%
    apart; compare adjacent runs only): session start 90.5-96.3 us with
    the fp8 hi/lo DR A-stage + bf16 natural copy (25.2 MiB, the v1
    design); rel err 4.07e-3 -> 6.14e-3 spent on byte/PE cuts.
"""

import numpy as np
import ml_dtypes

B, T, D, CTX = 64, 4096, 256, 100
NCORES = 8
BPC = B // NCORES          # samples per core
CP = 128                   # context dim padded to 128 (W/u zero-padded)
TSUP = 512                 # t-rows per supertile (one PSUM bank of xw)
BF16 = ml_dtypes.bfloat16
FP8 = ml_dtypes.float8_e4m3

_NC_CACHE: dict = {}


def build_nc(bpc=BPC, t_total=T):
    """Build (and cache) the Bass graph for one core's shard."""
    key = (bpc, t_total)
    if key in _NC_CACHE:
        return _NC_CACHE[key]

    from contextlib import ExitStack
    import concourse.bass as bass
    import concourse.tile as tile
    from concourse import bacc, mybir

    nsup = t_total // TSUP     # supertiles per sample (must be even)
    t_half = t_total // 2      # DMA slab = half a sample per layout
    nsup_h = nsup // 2         # supertiles per half-slab
    ns_h = t_half // 128       # t-rows per partition in one natural slab

    nc = bacc.Bacc("TRN2", target_bir_lowering=False, debug=False,
                   enable_asserts=False)
    ns_h_ = (t_total // 2) // 128
    xb = nc.declare_dram_parameter("xb", [bpc, 128, 2, ns_h_ // 4, D],
                                   mybir.dt.bfloat16, isOutput=False)
    x8n = nc.declare_dram_parameter("x8n", [bpc, 128, 2, 3 * ns_h_ // 4, D],
                                    mybir.dt.float8e4, isOutput=False)
    xt = nc.declare_dram_parameter("xt", [bpc, D, 2, t_half],
                                   mybir.dt.float8e4, isOutput=False)
    w = nc.declare_dram_parameter("w", [D, CP], mybir.dt.float8e4,
                                  isOutput=False)
    u = nc.declare_dram_parameter("u", [CP, 1], mybir.dt.bfloat16,
                                  isOutput=False)
    out = nc.declare_dram_parameter("out", [bpc, D], mybir.dt.float32,
                                    isOutput=True)

    FP32 = mybir.dt.float32
    BF = mybir.dt.bfloat16
    PSUM = bass.MemorySpace.PSUM
    AF = mybir.ActivationFunctionType

    with tile.TileContext(nc) as tc:
        with ExitStack() as ctx:
            const = ctx.enter_context(tc.tile_pool(name="const", bufs=1))
            xpool = ctx.enter_context(tc.tile_pool(name="x", bufs=8))
            xtpool = ctx.enter_context(tc.tile_pool(name="xt", bufs=3))
            thpool = ctx.enter_context(tc.tile_pool(name="th", bufs=4))
            ppool = ctx.enter_context(tc.tile_pool(name="p", bufs=12))
            fin = ctx.enter_context(tc.tile_pool(name="fin", bufs=4))
            xwps = ctx.enter_context(tc.tile_pool(name="xwps", bufs=2, space=PSUM))
            paps = ctx.enter_context(tc.tile_pool(name="paps", bufs=2, space=PSUM))
            oaps = ctx.enter_context(tc.tile_pool(name="oaps", bufs=2, space=PSUM))

            # State per sample, filled as the pipeline flows.
            xn = [None] * bpc
            xtt = [None] * bpc
            oacc = [None] * bpc      # [1, 260]: cols 0:256 out, 256 sum_p
            scols = [None] * bpc
            th = {}
            p_sb = {}

            # Head ordering on the sync queue: xtt0's h0 half (512 KiB,
            # feeds A pairs 0-1), then W (64 KiB), then the h1 half, then
            # the xn slab stream.  First A matmul gates on h0+W ~11 us.
            # xtt slabs for samples 1..7 ride the scalar HWDGE queue so
            # they never queue behind the 2x-bigger xn stream (v1 stall
            # mode) and their 8 issue slots barely load the ACT FIFO (the
            # v2 mistake was 16 xn issues there, which delayed xn behind
            # tanh/exp and stalled E every sample).
            def issue_xtt(bb):
                xtt[bb] = xtpool.tile([128, 2, 2, t_half],
                                      mybir.dt.float8e4,
                                      tag="xtt", name=f"xtt{bb}")
                nc.sync.dma_start(
                    xtt[bb][:],
                    xt[bb].rearrange("(c p) h t -> p c h t", p=128))

            # ALL slabs ride the single sync HWDGE ring, whose FIFO
            # delivers in issue order at full ring bandwidth; each
            # sample's xtt is issued TWO samples ahead of its xn slabs
            # (ring order: xtt0, W, xtt1, [xtt2 xn0] [xtt3 xn1] ...), so
            # the A stage never waits on a transposed slab.  (Split-ring
            # variants all starved one stream: packet round-robin gave
            # the xtt ring ~100 GB/s while sync carried xn, landing each
            # xtt ~3 us late.)
            xtt[0] = xtpool.tile([128, 2, 2, t_half], mybir.dt.float8e4,
                                 tag="xtt", name="xtt0")
            nc.sync.dma_start(
                xtt[0][:, :, 0, 0:2 * TSUP],
                xt[0, :, 0, 0:2 * TSUP].rearrange("(c p) t -> p c t",
                                                  p=128))
            w_sb = const.tile([128, 2, CP], mybir.dt.float8e4, tag="w")
            nc.sync.dma_start(w_sb[:], w.rearrange("(c p) m -> p c m", p=128))
            nc.sync.dma_start(
                xtt[0][:, :, 0, 2 * TSUP:t_half],
                xt[0, :, 0, 2 * TSUP:t_half].rearrange("(c p) t -> p c t",
                                                       p=128))
            nc.sync.dma_start(
                xtt[0][:, :, 1, :],
                xt[0, :, 1, :].rearrange("(c p) t -> p c t", p=128))
            issue_xtt(1)
            u_sb = const.tile([CP, 1], BF, tag="u")
            nc.scalar.dma_start(u_sb[:], u[:, :])
            onesf_sb = const.tile([128, 1], FP32, tag="onesf")
            nc.vector.memset(onesf_sb[:], 1.0)

            # Warm-up matmuls on a zeroed tile: keep the PE busy through
            # the DMA head so the HAM clock gate opens (4/8 -> 8/8 needs
            # ~3.4 us of sustained activity) before the first real matmul.
            # Many SMALL (N=128) warm-ups instead of few big ones: the
            # tail granularity is ~107 ns, so the first real matmul is
            # barely delayed when the first slab lands mid-warm-up.
            wsrc = const.tile([128, 512], BF, tag="wsrc")
            nc.vector.memset(wsrc[:], 0.0)
            warm = xwps.tile([128, 2, TSUP], FP32, tag="xw", name="warm")
            for _ in range(36):
                nc.tensor.matmul(warm[:, 0, 0:128], wsrc[:, 0:128],
                                 wsrc[:, 0:128], start=True, stop=True)

            def supt(g):
                return divmod(g, nsup)  # -> (sample, supertile-in-sample)

            def stage_A(g):
                """xw matmul pair + tanh for supertiles g, g+1."""
                b, st = supt(g)
                def issue_xn(bb):
                    # Natural copy, f=3/4 residual-compensated mixed
                    # precision: every 4th t-chunk ships bf16, carrying
                    # the fp8 residuals of its three partners; the rest
                    # ship fp8.  10.5 MiB instead of 16.8 per core, as
                    # ONE DMA per dtype per sample (big transfers keep
                    # the SDMA engines at line rate).
                    tb = xpool.tile([128, 2, ns_h // 4, D], BF,
                                    tag="xnb", name=f"xnb{bb}")
                    nc.sync.dma_start(tb[:], xb[bb])
                    t8 = xpool.tile([128, 2, 3 * ns_h // 4, D],
                                    mybir.dt.float8e4,
                                    tag="xn8", name=f"xn8{bb}")
                    nc.sync.dma_start(t8[:], x8n[bb])
                    xn[bb] = (tb, t8)

                if st == 0:
                    if b + 2 < bpc:
                        issue_xtt(b + 2)
                    issue_xn(b)
                    oacc[b] = oaps.tile([1, 260], FP32, tag="oacc",
                                        name=f"oacc{b}")
                    scols[b] = ppool.tile([128, nsup // 2], FP32,
                                          tag="scols", name=f"scols{b}")

                nq = 2
                # One 2-bank PSUM tile per pair; each supertile's matmuls
                # target their own bank (slice [:, i, :]), and ONE tanh
                # covers the pair ([128, 1024]): the ACT instruction's
                # ~352-cycle fixed cost is paid once, not twice (~9 us of
                # Scalar engine time across the kernel).
                xwp = xwps.tile([128, nq, TSUP], FP32, tag="xw",
                                name=f"xw{g}")
                # fp8 DoubleRow, hi-plane-only W: K=256 contracted in ONE
                # matmul per supertile at ~241 ns vs 2x213 for the bf16
                # two-half form.  The W e4m3 error is neutralized by
                # host-side ADAPTIVE ROUNDING: each row's fp8 rounding
                # directions are chosen so (dW @ u) ~ 0, which kills the
                # tanh-linearized logit error x.(dW u) (numpy: 1.44e-2
                # naive-rounded -> 5.75e-3 adaptive, = the exact-W case).
                DRM = mybir.MatmulPerfMode.DoubleRow
                for i in range(nq):
                    sti = st + i
                    h = sti // nsup_h
                    j0 = (sti % nsup_h) * TSUP
                    nc.tensor.matmul(xwp[:, i, :],
                                     w_sb[:],
                                     xtt[b][:, :, h, j0:j0 + TSUP],
                                     start=True, stop=True,
                                     perf_mode=DRM)
                # th in fp8: the C stage's per-logit LDWEIGHTS is its
                # wall-clock cost and FWL reads 4 fp8/cycle vs 2 bf16 --
                # halves the th load stream (~6.7 us of PE).  Accuracy
                # cost measured in numpy: 4.16e-3 -> 4.95e-3.
                thp = thpool.tile([128, nq, TSUP], mybir.dt.float8e4,
                                  tag="th", name=f"th{g}")
                nc.scalar.activation(thp[:], xwp[:], AF.Tanh,
                                     scale=1.0 / 256.0)
                for i in range(nq):
                    th[g + i] = thp[:, i, :]

            def stage_C(g0):
                """logits + exp + (DVE) partial sum_p for the supertile
                pair (g0, g0+1).  One [128, 8] PSUM group and ONE exp per
                pair: paps gets true double-buffering (bufs=2 over one
                tile/iteration instead of two), so the next pair's logits
                matmuls never wait on the previous exp.
                """
                b, st0 = supt(g0)
                pcc = paps.tile([128, 8], FP32, tag="pacc",
                                name=f"pacc{g0}")
                for j in range(2):
                    g = g0 + j
                    for s in range(4):
                        nc.tensor.matmul(pcc[:, 4 * j + s:4 * j + s + 1],
                                         th[g][:, s * 128:(s + 1) * 128],
                                         u_sb[:],
                                         start=(j == 0 and s == 0),
                                         stop=(j == 1 and s == 3))
                    del th[g]
                pp = ppool.tile([128, 8], BF, tag="p", name=f"p{g0}")
                # accum_out: the ACT engine's free-axis accumulator
                # produces sum_t p during the exp pass itself -- no DVE
                # reduce, and the C stage loses a cross-engine semaphore
                # dependency that stalled sample boundaries ~620 ns.
                nc.scalar.activation(pp[:], pcc[:], AF.Exp,
                                     accum_out=scols[b][:, st0 // 2:
                                                        st0 // 2 + 1])
                p_sb[g0 // 2] = pp

            def stage_E(g):
                """weighted-sum matmuls for supertile g (+ finalize)."""
                b, st = supt(g)
                pg, off = g // 2, (g % 2) * 4
                for s in range(4):
                    sg = 4 * st + s
                    h2, sl2 = sg // ns_h, sg % ns_h
                    if sl2 % 4 == 0:
                        rhs = xn[b][0][:, h2, sl2 // 4, :]
                    else:
                        rhs = xn[b][1][:, h2, sl2 - 1 - sl2 // 4, :]
                    nc.tensor.matmul(oacc[b][:, 0:D],
                                     p_sb[pg][:, off + s:off + s + 1],
                                     rhs,
                                     start=(sg == 0),
                                     stop=(sg == 4 * nsup - 1))
                if g % 2 == 1:
                    del p_sb[pg]
                if st == nsup - 1:
                    # Finalize sample b: out_row = oacc / sum_t p.  The
                    # scalar sum rides the spare PSUM columns of oacc.
                    s1v = fin.tile([128, 1], FP32, tag="s1v", name=f"s1v{b}")
                    nc.vector.reduce_sum(s1v[:], scols[b][:],
                                         axis=mybir.AxisListType.X)
                    nc.tensor.matmul(oacc[b][:, 256:257], onesf_sb[:],
                                     s1v[:])
                    rinv = fin.tile([1, 1], FP32, tag="rinv",
                                    name=f"rinv{b}")
                    nc.vector.reciprocal(rinv[:], oacc[b][:, 256:257])
                    osb = fin.tile([1, D], FP32, tag="osb", name=f"osb{b}")
                    nc.vector.tensor_scalar_mul(osb[:], oacc[b][:, 0:D],
                                                rinv[:])
                    # Out stores ride gpsimd's otherwise-idle SWDGE ring:
                    # on the sync ring their issue (which waits for the
                    # sample's finalize) blocked every xn slab issue
                    # queued behind it in FIFO order, throttling prefetch
                    # to ~1.5 samples (measured 52 us queue-head waits).
                    # The LAST two samples go on sync instead (no slab
                    # issues remain behind them, and HWDGE skips SWDGE's
                    # ~1 us Q7 descriptor setup -- trims the exit tail).
                    eng = nc.sync if b >= bpc - 2 else nc.gpsimd
                    eng.dma_start(out[b:b + 1, :], osb[:])

            # Pair-wise software pipeline over all supertiles of all
            # samples.  Per pair-iteration: E for supertiles 2pi-4/2pi-3
            # (lag 4: never waits on exp), C/D for 2pi-2/2pi-1, A/B for
            # 2pi/2pi+1.  PE work is emitted ready-first (E, C, A).
            ntot = bpc * nsup
            npair = ntot // 2
            for pi in range(npair + 2):
                for gg in (2 * pi - 4, 2 * pi - 3):
                    if 0 <= gg < ntot:
                        stage_E(gg)
                if 0 <= 2 * pi - 2 < ntot:
                    stage_C(2 * pi - 2)
                if pi < npair:
                    stage_A(2 * pi)

    nc.compile()
    _NC_CACHE[key] = nc
    return nc


def make_in_maps(X, W, u, ncores=NCORES):
    """Shard + cast the full inputs for the cores.

    xt is stored t-permuted: column j = s*128 + p holds X[t = NS*p + s, :],
    matching the natural slab's partition layout (see build_nc docstring).
    """
    Xf = np.asarray(X)
    bpc = Xf.shape[0] // ncores
    t_total = Xf.shape[1]
    ns = t_total // 128
    ns_h = ns // 2
    uu = np.asarray(u)[:, 0].astype(BF16).astype(np.float32)
    W256 = np.asarray(W, dtype=np.float32) * 256.0
    Wp = np.zeros((D, CP), dtype=FP8)
    Wp[:, :CTX] = _adaptive_round_w(W256, uu).astype(FP8)
    up = np.zeros((CP, 1), dtype=BF16)
    up[:CTX, :] = np.asarray(u).astype(BF16)
    in_maps = []
    for i in range(ncores):
        Xc = Xf[i * bpc:(i + 1) * bpc]
        # natural layout [b, h, p, s, d]: t = h*t_half + p*ns_h + s
        X5 = Xc.reshape(bpc, 2, 128, ns_h, D)
        # f=3/4 residual compensation: s-chunks not divisible by 4 ship
        # fp8; their quantization residuals are summed into the s%4==0
        # carrier chunk (adjacent timesteps, near-identical attention
        # weights) shipped in bf16.
        s8 = [s for s in range(ns_h) if s % 4]
        q8 = X5[:, :, :, s8, :].astype(FP8)
        resid = X5[:, :, :, s8, :] - q8.astype(np.float32)
        rsum = resid.reshape(bpc, 2, 128, ns_h // 4, 3, D).sum(axis=4)
        xbf = (X5[:, :, :, ::4, :] + rsum).astype(BF16)
        # device slabs are p-major: [b, p, h, s', d]
        xbf = np.ascontiguousarray(xbf.transpose(0, 2, 1, 3, 4))
        x8c = np.ascontiguousarray(q8.transpose(0, 2, 1, 3, 4))
        xs8 = Xc.astype(FP8)
        xts = np.ascontiguousarray(
            xs8.reshape(bpc, 2, 128, ns_h, D).transpose(0, 4, 1, 3, 2)
        ).reshape(bpc, D, 2, t_total // 2)
        in_maps.append({"xb": xbf, "x8n": x8c, "xt": xts, "w": Wp,
                        "u": up})
    return in_maps


def _adaptive_round_w(W256, uu):
    """fp8-e4m3 rounding of W256 with per-row direction choices greedily
    flipped so each row's error-dot-u, e_d = (Wq - W256)[d] @ u, is driven
    to ~0.  The dominant logit error from one-plane fp8 W is the
    tanh-linearized term x.(dW u); nulling dW@u kills it (measured 758x
    reduction in |e|, output rel err 1.44e-2 -> 5.75e-3)."""
    q = W256.astype(FP8).astype(np.float32)
    eps = np.where(q < W256, 1.0, -1.0).astype(np.float32)
    ulp = np.maximum(np.abs(q) * 2.0 ** -3, 2.0 ** -9)
    alt = (q + eps * ulp * 1.001).astype(FP8).astype(np.float32)
    Wq = q.copy()
    e = (Wq - W256) @ uu
    flip = (alt - q) * uu[None, :]
    nd = W256.shape[0]
    for _ in range(8):
        cand = e[:, None] + flip
        c = np.argmin(np.abs(cand), axis=1)
        val = cand[np.arange(nd), c]
        rows = np.where(np.abs(val) < np.abs(e) * 0.999)[0]
        if len(rows) == 0:
            break
        for dd in rows:
            cc = c[dd]
            Wq[dd, cc] = (alt[dd, cc] if Wq[dd, cc] == q[dd, cc]
                          else q[dd, cc])
            e[dd] = (Wq[dd] - W256[dd]) @ uu
            flip[dd, cc] = -flip[dd, cc]
    return Wq


# test.py sets _PROFILE=True to capture neuron-profile exec time here.
_PROFILE = False
LAST_RESULT = None


def kernel(X, W, u):
    global LAST_RESULT
    from concourse.bass_utils import run_bass_kernel_spmd

    nc = build_nc()
    in_maps = make_in_maps(X, W, u)
    res = run_bass_kernel_spmd(nc, in_maps, core_ids=list(range(NCORES)),
                               trace=_PROFILE)
    LAST_RESULT = res
    outs = [np.asarray(res.results[i]["out"], dtype=np.float32)
            for i in range(NCORES)]
    return np.concatenate(outs, axis=0)
